# revision 27
# baseline (speedup 1.0000x reference)
"""Multi-head dense GAT kernel for Trainium2 (8 NeuronCores, batch-parallel).

Problem: x:[8,1024,256] f32, adj:[8,1024,1024] int32{0,1},
         W:[8,64,256] f32 (per-head linear, [out,in]), a:[8,128] f32.
Reference: h = x@W_h^T; e_ij = leakyrelu(a1.h_i + a2.h_j, 0.2); mask adj==0;
           softmax over j; out = elu(attn@h); concat heads -> [8,1024,512].

Math used here (per batch b, head h; s_i = a1.h_i, t_j = a2.h_j, z = s_i+t_j):
  exp(leakyrelu(z)) = exp(z) * max(exp(-0.8 z), 1)
                    = e^{s_i} * max(e^{-0.8 s_i} e^{0.2 t_j}, e^{t_j})
  softmax is invariant to the per-row factor e^{s_i}, so the unnormalized
  masked weight is  S[j,i] = adj[i,j] * max(a'_i * bv_j, v_j)
  with a' = exp(-0.8 s), bv = exp(0.2 t), v = exp(t).
  out[i,:] = elu( (sum_j S[j,i] h[j,:]) / (sum_j S[j,i]) ).
  Also s = x @ (W^T a1), t = x @ (W^T a2) (associativity), so h is only
  needed for the final weighted sum.

Sharding: batch-parallel, core c computes batch element c.
"""

import os
import numpy as np
import ml_dtypes

B, N, D = 8, 1024, 256
H, K = 8, 64
NCORES = 8
P = 128
NJT = N // P          # 8 j-tiles
NIC = N // P          # 8 i-chunks
HPAIRS = H // 2

_CACHED = {}


def _build_nc():
    import concourse.bass as bass
    import concourse.mybir as mybir
    import concourse.tile as tile
    from concourse import bacc
    from concourse.masks import make_identity

    dt = mybir.dt
    Alu = mybir.AluOpType
    Act = mybir.ActivationFunctionType
    AP = bass.AP

    nc = bacc.Bacc(None, target_bir_lowering=False, debug=False)

    # ---- DRAM I/O (per-core shard) ----
    xT = nc.dram_tensor("xT", [D, N], dt.float32, kind="ExternalInput")
    adjT = nc.dram_tensor("adjT", [N, N], dt.bfloat16, kind="ExternalInput")
    w = nc.dram_tensor("w", [H, K, D], dt.float32, kind="ExternalInput")
    wT = nc.dram_tensor("wT", [P, 2, H, K], dt.float32, kind="ExternalInput")
    aT = nc.dram_tensor("aT", [K, H, 2], dt.float32, kind="ExternalInput")
    outT = nc.dram_tensor("outT", [H, N, K], dt.float32, kind="ExternalOutput")

    debug = bool(int(os.environ.get("GAT_DEBUG", "0")))
    if debug:
        dbg_ex = nc.dram_tensor("dbg_ex", [2, 16, N], dt.float32, kind="ExternalOutput")
        dbg_vt = nc.dram_tensor("dbg_vt", [P, NJT, 16], dt.float32, kind="ExternalOutput")
        dbg_abc = nc.dram_tensor("dbg_abc", [P, H, N], dt.bfloat16, kind="ExternalOutput")
        dbg_S = nc.dram_tensor("dbg_S", [P, 2, NJT, N], dt.bfloat16, kind="ExternalOutput")
        dbg_hext = nc.dram_tensor("dbg_hext", [P, NJT, H * 65], dt.bfloat16, kind="ExternalOutput")
        dbg_stage = nc.dram_tensor("dbg_stage", [P, 1024], dt.float32, kind="ExternalOutput")

    with tile.TileContext(nc) as tc:
        with (
            tc.tile_pool(name="const", bufs=1) as constp,
            tc.tile_pool(name="prep", bufs=1) as prep,
            tc.tile_pool(name="big", bufs=1) as big,
            tc.tile_pool(name="spool", bufs=2) as spool,
            tc.tile_pool(name="tp", bufs=3) as tp,
            tc.tile_pool(name="ep", bufs=2) as ep,
            tc.tile_pool(name="po", bufs=4, space="PSUM") as pop,
        ):
            ident = constp.tile([P, P], dt.float32)
            make_identity(nc, ident)
            ones1 = constp.tile([1, P], dt.float32)
            nc.vector.memset(ones1[:], 1.0)
            zb = constp.tile([P, 1], dt.float32)
            nc.vector.memset(zb[:], 0.0)
            m1b = constp.tile([P, 1], dt.float32)
            nc.vector.memset(m1b[:], -1.0)

            # ---- load inputs (ordered by dependency criticality) ----
            w_sb = prep.tile([K, H, D], dt.float32)
            nc.sync.dma_start(w_sb[:], w[:].rearrange("h k d -> k h d"))
            a_sb = prep.tile([K, H, 2], dt.float32)
            nc.sync.dma_start(a_sb[:], aT[:])
            xt_sb = prep.tile([P, 2, N], dt.float32)       # xT d-chunks
            nc.sync.dma_start(xt_sb[:], xT[:].rearrange("(c p) n -> p c n", p=P))
            mT = big.tile([P, NJT, N], dt.bfloat16)        # transposed adj mask
            adjT_r = adjT[:].rearrange("(t p) i -> p t i", p=P)
            nc.sync.dma_start(mT[:, 0, :], adjT_r[:, 0, :])
            wt_sb = prep.tile([P, 2, H, K], dt.float32)
            nc.sync.dma_start(wt_sb[:], wT[:])
            for jt in range(1, NJT):
                nc.sync.dma_start(mT[:, jt, :], adjT_r[:, jt, :])

            # ---- wtilde = W_h^T @ [a1|a2]; psum col c*16 + half*8 + h ----
            ps_w = pop.tile([P, 32], dt.float32, tag="po")
            for h in range(H):
                for c in range(2):
                    for half in range(2):
                        nc.tensor.matmul(
                            ps_w[:, c * 16 + half * 8 + h : c * 16 + half * 8 + h + 1],
                            w_sb[:, h, c * P : (c + 1) * P],
                            a_sb[:, h, half : half + 1],
                            start=True, stop=True,
                        )
            wt2_sb = prep.tile([P, 32], dt.float32)
            nc.vector.tensor_copy(wt2_sb[:], ps_w[:])

            # ---- s_self rows 0-7, s_nb rows 0-7 (separate psums) ----
            ps_ss = pop.tile([8, N], dt.float32, tag="po")
            ps_sn = pop.tile([8, N], dt.float32, tag="po")
            for half in range(2):
                for c in range(2):
                    nc.tensor.matmul(
                        ps_ss[:, half * 512 : (half + 1) * 512],
                        wt2_sb[:, c * 16 : c * 16 + 8],
                        xt_sb[:, c, half * 512 : (half + 1) * 512],
                        start=(c == 0), stop=(c == 1),
                    )
            for half in range(2):
                for c in range(2):
                    nc.tensor.matmul(
                        ps_sn[:, half * 512 : (half + 1) * 512],
                        wt2_sb[:, c * 16 + 8 : c * 16 + 16],
                        xt_sb[:, c, half * 512 : (half + 1) * 512],
                        start=(c == 0), stop=(c == 1),
                    )

            # ---- exp vectors: a' = e^{-0.8 s}, bv = e^{0.2 t}, v = e^{t} ----
            exS = prep.tile([8, N], dt.bfloat16)
            exBV = prep.tile([8, N], dt.float32)
            exVV = prep.tile([8, N], dt.float32)
            nc.scalar.activation(exS[:], ps_ss[:], Act.Exp, bias=zb[:8, :], scale=-0.8)
            nc.scalar.activation(exBV[:], ps_sn[:], Act.Exp, bias=zb[:8, :], scale=0.2)
            nc.scalar.activation(exVV[:], ps_sn[:], Act.Exp, bias=zb[:8, :], scale=1.0)

            # ---- vt: per j-tile transposed scalar columns [128, 16]
            #      col h = bv_h[j], col 8+h = v_h[j]
            vt_sb = prep.tile([P, NJT, 16], dt.float32)
            for jt in range(NJT):
                ps_vt = pop.tile([P, 16], dt.float32, tag="po")
                nc.tensor.transpose(ps_vt[:, 0:8], exBV[:, jt * P : (jt + 1) * P], ident[:8, :8])
                nc.tensor.transpose(ps_vt[:, 8:16], exVV[:, jt * P : (jt + 1) * P], ident[:8, :8])
                nc.vector.tensor_copy(vt_sb[:, jt, :], ps_vt[:])

            # ---- a'-broadcast: exS rows -> DRAM (bf16), then one
            #      broadcast-read DMA per head pair (ACT HWDGE ring) ----
            aScr = nc.dram_tensor("aScr", [H, N], dt.bfloat16, kind="Internal")
            nc.scalar.dma_start(aScr[:], exS[:])
            abc = big.tile([P, H, N], dt.bfloat16)
            for hp in range(HPAIRS):
                nc.scalar.dma_start(
                    abc[:, 2 * hp : 2 * hp + 2, :],
                    AP(aScr[:].tensor, 2 * hp * N, [[0, P], [N, 2], [1, N]]),
                )

            # ---- h-ext per j-tile: [128, H*65] bf16, col h*65+64 stays 1.0 ----
            hext = big.tile([P, NJT, H * 65], dt.bfloat16)
            nc.gpsimd.memset(hext[:], 1.0)
            for jt in range(NJT):
                ps_h = pop.tile([P, 512], dt.float32, tag="po")
                for c in range(2):
                    nc.tensor.matmul(
                        ps_h[:, :],
                        xt_sb[:, c, jt * P : (jt + 1) * P],
                        wt_sb[:, c, :, :],
                        start=(c == 0), stop=(c == 1),
                    )
                nc.scalar.copy(
                    hext[:, jt, :].rearrange("p (h k) -> p h k", h=H)[:, :, 0:K],
                    ps_h[:].rearrange("p (h k) -> p h k", h=H),
                )

            if debug:
                nc.gpsimd.dma_start(dbg_ex[0][0:8], exS[:])
                nc.sync.dma_start(dbg_ex[0][8:16], exBV[:])
                nc.sync.dma_start(dbg_ex[1][0:8], exVV[:])
                nc.sync.dma_start(dbg_vt[:], vt_sb[:])
                nc.sync.dma_start(dbg_abc[:], abc[:])
                nc.sync.dma_start(dbg_hext[:], hext[:])

            # ---- main loop over head pairs (epilogue deferred one pair) ----
            def s_pass(hp, S):
                h0 = 2 * hp
                for jt in range(NJT):
                    t2 = tp.tile([P, 2, N], dt.bfloat16, tag="t2")
                    for hh in range(2):
                        h = h0 + hh
                        nc.vector.tensor_scalar(
                            t2[:, hh, :],
                            abc[:, h, :],
                            vt_sb[:, jt, h : h + 1],
                            vt_sb[:, jt, 8 + h : 8 + h + 1],
                            Alu.mult,
                            Alu.max,
                        )
                    mTb = mT[:, jt, :]
                    nc.vector.tensor_tensor(
                        S[:, :, jt, :],
                        t2[:],
                        AP(mTb.tensor, mTb.offset, [mTb.ap[0], [0, 2], [1, N]]),
                        Alu.mult,
                    )

            def mms(hp, hh, S, ps_o):
                h = 2 * hp + hh
                for ic in range(NIC):
                    off = (ic // 4) * 512 + (ic % 4) * 65
                    for jt in range(NJT):
                        nc.tensor.matmul(
                            ps_o[:, off : off + 65],
                            S[:, hh, jt, ic * P : (ic + 1) * P],
                            hext[:, jt, h * 65 : (h + 1) * 65],
                            start=(jt == 0), stop=(jt == NJT - 1),
                        )

            def epilogue(hp, hh, ps_o):
                h = 2 * hp + hh
                rec = ep.tile([P, 8], dt.float32, tag="rec")
                nc.vector.reciprocal(
                    rec[:].rearrange("p (b q) -> p b q", b=2),
                    AP(ps_o.tensor, ps_o.offset + 64, [[1024, P], [512, 2], [65, 4]]),
                )
                stage = ep.tile([P, 512], dt.float32, tag="stage")
                nc.vector.tensor_tensor(
                    stage[:].rearrange("p (b q k) -> p b q k", b=2, q=4),
                    AP(ps_o.tensor, ps_o.offset, [[1024, P], [512, 2], [65, 4], [1, K]]),
                    AP(rec.tensor, rec.offset, [[8, P], [4, 2], [1, 4], [0, K]]),
                    Alu.mult,
                )
                if debug and hp == 0:
                    nc.sync.dma_start(dbg_stage[:, hh * 512 : (hh + 1) * 512], stage[:])
                # elu(y) = relu(y) + exp(min(y,0)) - 1
                r1 = ep.tile([P, 512], dt.float32, tag="r1")
                nc.scalar.activation(r1[:], stage[:], Act.Relu, bias=zb[:], scale=-1.0)
                nc.scalar.activation(r1[:], r1[:], Act.Exp, bias=zb[:], scale=-1.0)
                nc.scalar.activation(r1[:], r1[:], Act.Identity, bias=m1b[:])
                nc.vector.scalar_tensor_tensor(
                    stage[:], stage[:], 0.0, r1[:], Alu.max, Alu.add,
                )
                nc.scalar.dma_start(
                    outT[h].rearrange("(ic p) k -> p ic k", p=P),
                    stage[:].rearrange("p (ic k) -> p ic k", ic=NIC),
                )

            pending = []
            for hp in range(HPAIRS):
                S = spool.tile([P, 2, NJT, N], dt.bfloat16, tag="S")
                s_pass(hp, S)
                if debug and hp == 0:
                    nc.sync.dma_start(dbg_S[:], S[:])
                for hh in range(2):
                    ps_o = pop.tile([P, 1024], dt.float32, tag="po")
                    mms(hp, hh, S, ps_o)
                    if len(pending) >= 2:
                        epilogue(*pending.pop(0))
                    pending.append((hp, hh, ps_o))
            for args in pending:
                epilogue(*args)

    nc.finalize()
    return nc


def _get_nc():
    if "nc" not in _CACHED:
        _CACHED["nc"] = _build_nc()
    return _CACHED["nc"]


def kernel(x, adj, W, a):
    from concourse.bass_utils import run_bass_kernel_spmd

    x = np.asarray(x)
    adj = np.asarray(adj)
    W = np.asarray(W, dtype=np.float32)
    a = np.asarray(a, dtype=np.float32)

    wT_host = np.ascontiguousarray(W.reshape(H, K, 2, P).transpose(3, 2, 0, 1))
    aT_host = np.ascontiguousarray(a.reshape(H, 2, K).transpose(2, 0, 1))

    in_maps = []
    for c in range(NCORES):
        in_maps.append({
            "xT": np.ascontiguousarray(x[c].T.astype(np.float32)),
            "adjT": np.ascontiguousarray(adj[c].T.astype(ml_dtypes.bfloat16)),
            "w": W,
            "wT": wT_host,
            "aT": aT_host,
        })

    nc = _get_nc()
    res = run_bass_kernel_spmd(
        nc, in_maps, core_ids=list(range(NCORES)),
        trace=bool(int(os.environ.get("GAT_TRACE", "0"))),
    )
    _CACHED["last_results"] = res

    out = np.empty((B, N, H * K), dtype=np.float32)
    for c in range(NCORES):
        oT = res.results[c]["outT"]            # [H, N, K]
        out[c] = oT.transpose(1, 0, 2).reshape(N, H * K)
    return out


# revision 31
# speedup vs baseline: 1.0416x; 1.0416x over previous
"""Multi-head dense GAT kernel for Trainium2 (8 NeuronCores, batch-parallel).

Problem: x:[8,1024,256] f32, adj:[8,1024,1024] int32{0,1},
         W:[8,64,256] f32 (per-head linear, [out,in]), a:[8,128] f32.
Reference: h = x@W_h^T; e_ij = leakyrelu(a1.h_i + a2.h_j, 0.2); mask adj==0;
           softmax over j; out = elu(attn@h); concat heads -> [8,1024,512].

Math used here (per batch b, head h; s_i = a1.h_i, t_j = a2.h_j, z = s_i+t_j):
  exp(leakyrelu(z)) = exp(z) * max(exp(-0.8 z), 1)
                    = e^{s_i} * max(e^{-0.8 s_i} e^{0.2 t_j}, e^{t_j})
  softmax is invariant to the per-row factor e^{s_i}, so the unnormalized
  masked weight is  S[j,i] = adj[i,j] * max(a'_i * bv_j, v_j)
  with a' = exp(-0.8 s), bv = exp(0.2 t), v = exp(t).
  out[i,:] = elu( (sum_j S[j,i] h[j,:]) / (sum_j S[j,i]) ).
  Also s = x @ (W^T a1), t = x @ (W^T a2) (associativity), so h is only
  needed for the final weighted sum.

Sharding: batch-parallel, core c computes batch element c.
"""

import os
import numpy as np
import ml_dtypes

B, N, D = 8, 1024, 256
H, K = 8, 64
NCORES = 8
P = 128
NJT = N // P          # 8 j-tiles
NIC = N // P          # 8 i-chunks
HPAIRS = H // 2

_CACHED = {}


def _build_nc():
    import concourse.bass as bass
    import concourse.mybir as mybir
    import concourse.tile as tile
    from concourse import bacc
    from concourse.masks import make_identity

    dt = mybir.dt
    Alu = mybir.AluOpType
    Act = mybir.ActivationFunctionType
    AP = bass.AP

    nc = bacc.Bacc(None, target_bir_lowering=False, debug=False)

    # ---- DRAM I/O (per-core shard) ----
    xT = nc.dram_tensor("xT", [D, N], dt.float32, kind="ExternalInput")
    adjT = nc.dram_tensor("adjT", [N, N], dt.bfloat16, kind="ExternalInput")
    w = nc.dram_tensor("w", [H, K, D], dt.float32, kind="ExternalInput")
    wT = nc.dram_tensor("wT", [P, 2, H, K], dt.float32, kind="ExternalInput")
    aT = nc.dram_tensor("aT", [K, H, 2], dt.float32, kind="ExternalInput")
    outT = nc.dram_tensor("outT", [H, N, K], dt.float32, kind="ExternalOutput")

    debug = bool(int(os.environ.get("GAT_DEBUG", "0")))
    if debug:
        dbg_ex = nc.dram_tensor("dbg_ex", [2, 16, N], dt.float32, kind="ExternalOutput")
        dbg_vt = nc.dram_tensor("dbg_vt", [P, NJT, 16], dt.float32, kind="ExternalOutput")
        dbg_abc = nc.dram_tensor("dbg_abc", [P, H, N], dt.bfloat16, kind="ExternalOutput")
        dbg_S = nc.dram_tensor("dbg_S", [P, 2, NJT, N], dt.bfloat16, kind="ExternalOutput")
        dbg_hext = nc.dram_tensor("dbg_hext", [P, NJT, H * 65], dt.bfloat16, kind="ExternalOutput")
        dbg_stage = nc.dram_tensor("dbg_stage", [P, 1024], dt.float32, kind="ExternalOutput")

    with tile.TileContext(nc) as tc:
        with (
            tc.tile_pool(name="const", bufs=1) as constp,
            tc.tile_pool(name="prep", bufs=1) as prep,
            tc.tile_pool(name="big", bufs=1) as big,
            tc.tile_pool(name="spool", bufs=2) as spool,
            tc.tile_pool(name="tp", bufs=3) as tp,
            tc.tile_pool(name="ep", bufs=4) as ep,
            tc.tile_pool(name="po", bufs=4, space="PSUM") as pop,
        ):
            ident = constp.tile([P, P], dt.float32)
            make_identity(nc, ident)
            ones1 = constp.tile([1, P], dt.float32)
            nc.vector.memset(ones1[:], 1.0)
            zb = constp.tile([P, 1], dt.float32)
            nc.vector.memset(zb[:], 0.0)
            m1b = constp.tile([P, 1], dt.float32)
            nc.vector.memset(m1b[:], -1.0)

            # ---- load inputs (ordered by dependency criticality) ----
            w_sb = prep.tile([K, H, D], dt.float32)
            nc.sync.dma_start(w_sb[:], w[:].rearrange("h k d -> k h d"))
            a_sb = prep.tile([K, H, 2], dt.float32)
            nc.sync.dma_start(a_sb[:], aT[:])
            xt_sb = prep.tile([P, 2, N], dt.float32)       # xT d-chunks
            nc.sync.dma_start(xt_sb[:], xT[:].rearrange("(c p) n -> p c n", p=P))
            mT = big.tile([P, NJT, N], dt.bfloat16)        # transposed adj mask
            adjT_r = adjT[:].rearrange("(t p) i -> p t i", p=P)
            nc.sync.dma_start(mT[:, 0, :], adjT_r[:, 0, :])
            wt_sb = prep.tile([P, 2, H, K], dt.float32)
            nc.sync.dma_start(wt_sb[:], wT[:])
            for jt in range(1, NJT):
                nc.sync.dma_start(mT[:, jt, :], adjT_r[:, jt, :])

            # ---- wtilde = W_h^T @ [a1|a2]; psum col c*16 + half*8 + h ----
            ps_w = pop.tile([P, 32], dt.float32, tag="po")
            for h in range(H):
                for c in range(2):
                    for half in range(2):
                        nc.tensor.matmul(
                            ps_w[:, c * 16 + half * 8 + h : c * 16 + half * 8 + h + 1],
                            w_sb[:, h, c * P : (c + 1) * P],
                            a_sb[:, h, half : half + 1],
                            start=True, stop=True,
                        )
            wt2_sb = prep.tile([P, 32], dt.float32)
            nc.vector.tensor_copy(wt2_sb[:], ps_w[:])

            # ---- s_self rows 0-7, s_nb rows 0-7 (separate psums) ----
            ps_ss = pop.tile([8, N], dt.float32, tag="po")
            ps_sn = pop.tile([8, N], dt.float32, tag="po")
            for half in range(2):
                for c in range(2):
                    nc.tensor.matmul(
                        ps_ss[:, half * 512 : (half + 1) * 512],
                        wt2_sb[:, c * 16 : c * 16 + 8],
                        xt_sb[:, c, half * 512 : (half + 1) * 512],
                        start=(c == 0), stop=(c == 1),
                    )
            for half in range(2):
                for c in range(2):
                    nc.tensor.matmul(
                        ps_sn[:, half * 512 : (half + 1) * 512],
                        wt2_sb[:, c * 16 + 8 : c * 16 + 16],
                        xt_sb[:, c, half * 512 : (half + 1) * 512],
                        start=(c == 0), stop=(c == 1),
                    )

            # ---- exp vectors: a' = e^{-0.8 s}, bv = e^{0.2 t}, v = e^{t} ----
            exS = prep.tile([8, N], dt.bfloat16)
            exBV = prep.tile([8, N], dt.float32)
            exVV = prep.tile([8, N], dt.float32)
            nc.scalar.activation(exS[:], ps_ss[:], Act.Exp, bias=zb[:8, :], scale=-0.8)
            nc.scalar.activation(exBV[:], ps_sn[:], Act.Exp, bias=zb[:8, :], scale=0.2)
            nc.scalar.activation(exVV[:], ps_sn[:], Act.Exp, bias=zb[:8, :], scale=1.0)

            # ---- vt: per j-tile transposed scalar columns [128, 16]
            #      col h = bv_h[j], col 8+h = v_h[j]
            vt_sb = prep.tile([P, NJT, 16], dt.float32)
            for jt in range(NJT):
                ps_vt = pop.tile([P, 16], dt.float32, tag="po")
                nc.tensor.transpose(ps_vt[:, 0:8], exBV[:, jt * P : (jt + 1) * P], ident[:8, :8])
                nc.tensor.transpose(ps_vt[:, 8:16], exVV[:, jt * P : (jt + 1) * P], ident[:8, :8])
                nc.vector.tensor_copy(vt_sb[:, jt, :], ps_vt[:])

            # ---- a'-broadcast: exS rows -> DRAM (bf16), then one
            #      broadcast-read DMA per head pair (ACT HWDGE ring) ----
            aScr = nc.dram_tensor("aScr", [H, N], dt.bfloat16, kind="Internal")
            nc.scalar.dma_start(aScr[:], exS[:])
            abc = big.tile([P, H, N], dt.bfloat16)
            for hp in range(HPAIRS):
                nc.scalar.dma_start(
                    abc[:, 2 * hp : 2 * hp + 2, :],
                    AP(aScr[:].tensor, 2 * hp * N, [[0, P], [N, 2], [1, N]]),
                )

            # ---- h-ext per j-tile: [128, H*65] bf16, col h*65+64 stays 1.0 ----
            hext = big.tile([P, NJT, H * 65], dt.bfloat16)
            nc.gpsimd.memset(hext[:], 1.0)
            for jt in range(NJT):
                ps_h = pop.tile([P, 512], dt.float32, tag="po")
                for c in range(2):
                    nc.tensor.matmul(
                        ps_h[:, :],
                        xt_sb[:, c, jt * P : (jt + 1) * P],
                        wt_sb[:, c, :, :],
                        start=(c == 0), stop=(c == 1),
                    )
                nc.scalar.copy(
                    hext[:, jt, :].rearrange("p (h k) -> p h k", h=H)[:, :, 0:K],
                    ps_h[:].rearrange("p (h k) -> p h k", h=H),
                )

            if debug:
                nc.gpsimd.dma_start(dbg_ex[0][0:8], exS[:])
                nc.sync.dma_start(dbg_ex[0][8:16], exBV[:])
                nc.sync.dma_start(dbg_ex[1][0:8], exVV[:])
                nc.sync.dma_start(dbg_vt[:], vt_sb[:])
                nc.sync.dma_start(dbg_abc[:], abc[:])
                nc.sync.dma_start(dbg_hext[:], hext[:])

            # ---- main loop over head pairs (epilogue deferred one pair) ----
            def s_pass(hp, S):
                h0 = 2 * hp
                for jt in range(NJT):
                    t2 = tp.tile([P, 2, N], dt.bfloat16, tag="t2")
                    for hh in range(2):
                        h = h0 + hh
                        nc.vector.tensor_scalar(
                            t2[:, hh, :],
                            abc[:, h, :],
                            vt_sb[:, jt, h : h + 1],
                            vt_sb[:, jt, 8 + h : 8 + h + 1],
                            Alu.mult,
                            Alu.max,
                        )
                    mTb = mT[:, jt, :]
                    nc.vector.tensor_tensor(
                        S[:, :, jt, :],
                        t2[:],
                        AP(mTb.tensor, mTb.offset, [mTb.ap[0], [0, 2], [1, N]]),
                        Alu.mult,
                    )

            def mms(hp, hh, S, ps_o):
                h = 2 * hp + hh
                for ic in range(NIC):
                    off = (ic // 4) * 512 + (ic % 4) * 65
                    for jt in range(NJT):
                        nc.tensor.matmul(
                            ps_o[:, off : off + 65],
                            S[:, hh, jt, ic * P : (ic + 1) * P],
                            hext[:, jt, h * 65 : (h + 1) * 65],
                            start=(jt == 0), stop=(jt == NJT - 1),
                        )

            def epi1(hp, hh, ps_o):
                h = 2 * hp + hh
                rec = ep.tile([P, 8], dt.float32, tag="rec")
                nc.vector.reciprocal(
                    rec[:].rearrange("p (b q) -> p b q", b=2),
                    AP(ps_o.tensor, ps_o.offset + 64, [[1024, P], [512, 2], [65, 4]]),
                )
                stage = ep.tile([P, 512], dt.float32, tag="stage")
                nc.vector.tensor_tensor(
                    stage[:].rearrange("p (b q k) -> p b q k", b=2, q=4),
                    AP(ps_o.tensor, ps_o.offset, [[1024, P], [512, 2], [65, 4], [1, K]]),
                    AP(rec.tensor, rec.offset, [[8, P], [4, 2], [1, 4], [0, K]]),
                    Alu.mult,
                )
                if debug and hp == 0:
                    nc.sync.dma_start(dbg_stage[:, hh * 512 : (hh + 1) * 512], stage[:])
                # elu(y) = relu(y) + exp(min(y,0)) - 1; ACT part here, DVE
                # combine deferred (phase 2) so it never stalls on ACT
                r1 = ep.tile([P, 512], dt.float32, tag="r1")
                nc.scalar.activation(r1[:], stage[:], Act.Relu, bias=zb[:], scale=-1.0)
                nc.scalar.activation(r1[:], r1[:], Act.Exp, bias=zb[:], scale=-1.0)
                nc.scalar.activation(r1[:], r1[:], Act.Identity, bias=m1b[:])
                return (h, stage, r1)

            def epi2(h, stage, r1):
                nc.vector.scalar_tensor_tensor(
                    stage[:], stage[:], 0.0, r1[:], Alu.max, Alu.add,
                )
                nc.scalar.dma_start(
                    outT[h].rearrange("(ic p) k -> p ic k", p=P),
                    stage[:].rearrange("p (ic k) -> p ic k", ic=NIC),
                )

            pend1 = []
            pend2 = []
            for hp in range(HPAIRS):
                S = spool.tile([P, 2, NJT, N], dt.bfloat16, tag="S")
                s_pass(hp, S)
                if debug and hp == 0:
                    nc.sync.dma_start(dbg_S[:], S[:])
                for hh in range(2):
                    ps_o = pop.tile([P, 1024], dt.float32, tag="po")
                    mms(hp, hh, S, ps_o)
                    if len(pend1) >= 2:
                        pend2.append(epi1(*pend1.pop(0)))
                    if len(pend2) >= 1:
                        epi2(*pend2.pop(0))
                    pend1.append((hp, hh, ps_o))
            for args in pend1:
                pend2.append(epi1(*args))
            for args in pend2:
                epi2(*args)

    nc.finalize()
    return nc


def _get_nc():
    if "nc" not in _CACHED:
        _CACHED["nc"] = _build_nc()
    return _CACHED["nc"]


def kernel(x, adj, W, a):
    from concourse.bass_utils import run_bass_kernel_spmd

    x = np.asarray(x)
    adj = np.asarray(adj)
    W = np.asarray(W, dtype=np.float32)
    a = np.asarray(a, dtype=np.float32)

    wT_host = np.ascontiguousarray(W.reshape(H, K, 2, P).transpose(3, 2, 0, 1))
    aT_host = np.ascontiguousarray(a.reshape(H, 2, K).transpose(2, 0, 1))

    in_maps = []
    for c in range(NCORES):
        in_maps.append({
            "xT": np.ascontiguousarray(x[c].T.astype(np.float32)),
            "adjT": np.ascontiguousarray(adj[c].T.astype(ml_dtypes.bfloat16)),
            "w": W,
            "wT": wT_host,
            "aT": aT_host,
        })

    nc = _get_nc()
    res = run_bass_kernel_spmd(
        nc, in_maps, core_ids=list(range(NCORES)),
        trace=bool(int(os.environ.get("GAT_TRACE", "0"))),
    )
    _CACHED["last_results"] = res

    out = np.empty((B, N, H * K), dtype=np.float32)
    for c in range(NCORES):
        oT = res.results[c]["outT"]            # [H, N, K]
        out[c] = oT.transpose(1, 0, 2).reshape(N, H * K)
    return out
